# revision 102
# baseline (speedup 1.0000x reference)
"""Trainium2 Bass kernel for nn_DGCNN6_homo (DGCNN with homophily features).

Strategy: shard the B=32 graphs across 8 NeuronCores (4 graphs/core, SPMD).
Per graph, each DynamicEdgeConv is computed as:
  - scores v_ij = x_i . x_j - s_j/2 via one augmented PE matmul (row of -1 /
    s/2 appended to the operands), so kNN ranking needs no elementwise pass.
    Feature tiles are fp16 so every big matmul runs at 1 cycle/row on the PE.
  - top-5 selection: a 3-round pairwise-max tournament (f16 tensor_tensor,
    which unlike Max8 runs in the 2x DVE mode) reduces each row's 2048
    scores to 256 comb maxima, max8 ranks those, and one max_index scan of
    the full score tile recovers exact indices. ~4% of rows may swap their
    5th neighbor for the 6th when two true top-5 share a comb (j mod 256) --
    well inside the output tolerance.
  - neighbor features gathered with the SWDGE dma_gather (B_j = x_j W1b rows,
    token-major fp16), transposed back to feature-major on the PE while
    accumulating A_i = x_i (W1a - W1b) into the same PSUM tile
  - per-edge 2-layer MLP on PE/ACT (leaky relu via Lrelu activation),
    neighbor aggregation as f16 adds on the Pool engine
  - lin1 + mean-pool reordered as (sum_nodes Lrelu(feat @ l1w1 + b1)) @ l1w2
    using the ACT accumulator, so the second matmul runs on [512]-vectors.
The 12 (conv, graph) instances run as ONE flat software pipeline over global
chunks so the DVE (the bottleneck engine) never idles at instance
boundaries; prep chains (srow/atab/btab) and per-512-node lin1 pieces are
drip-fed between chunks as side work, a slice of the score staging copies
(block 3 of alternate chunks; every chunk during conv3, where lin1 loads
the ACT engine) runs on the DVE to balance it against the ACT engine, and
the last graph's lin1 reuses the score PSUM ring after the final selection
chunk.
Homophily (equal-value fraction over the k=50 kNN graph) is exactly zero
unless a graph has duplicate coordinate values; the rare duplicates are
counted exactly on the host (a few rows of distance computation at most) and
enter the device's final MLP as 4 scalars per graph.
"""
import numpy as np

N = 2048          # nodes per graph
B_TOTAL = 32      # graphs
G = 4             # graphs per core
NB = 16           # 128-row blocks per graph
KEC = 5           # edge-conv k
NCORES = 8

_MODULE = None


# --------------------------------------------------------------------------
# device module
# --------------------------------------------------------------------------
def _build_module():
    import os
    CB = int(os.environ.get("K_CB", "4"))
    NCH = NB // CB
    WPAD = 64 if CB > 6 else 32
    NINST = 3 * G
    import concourse.bacc as bacc
    import concourse.mybir as mybir
    from concourse.tile import TileContext
    from concourse.masks import make_identity

    fp32 = mybir.dt.float32
    f16 = mybir.dt.float16
    i16 = mybir.dt.int16
    u16 = mybir.dt.uint16
    AF = mybir.ActivationFunctionType
    ALU = mybir.AluOpType
    AX = mybir.AxisListType

    nc = bacc.Bacc(None, target_bir_lowering=False, debug=False)

    def win(name, shape, dt=fp32):
        return nc.dram_tensor(name, shape, dt, kind="ExternalInput")

    xx_in = win("xx", [G * N, 4])
    c1w1d = win("c1w1d", [4, 64], f16);  c1w1b = win("c1w1b", [4, 128], f16)
    c1b1 = win("c1b1", [64, 1]);  c1w2 = win("c1w2", [64, 64], f16)
    c1b2 = win("c1b2", [64, 1])
    c2w1d = win("c2w1d", [64, 128], f16);  c2w1b = win("c2w1b", [64, 128], f16)
    c2b1 = win("c2b1", [128, 1]);  c2w2 = win("c2w2", [128, 64], f16)
    c2b2 = win("c2b2", [64, 1])
    l1x1 = win("l1x1", [64, 512], f16);  l1x2 = win("l1x2", [64, 512], f16)
    l1x3 = win("l1x3", [68, 512], f16)
    l1b1 = win("l1b1", [128, 4])
    l1w2 = win("l1w2", [128, 4, 256], f16);  l1b2 = win("l1b2", [128, 2])
    mw1 = win("mw1", [128, 3, 256]);  mb1 = win("mb1", [128, 2])
    mw2 = win("mw2", [128, 2, 3]);  mb2 = win("mb2", [3, 1])
    hvals = win("hvals", [4, G])
    repeye = win("repeye", [16, 128])

    outT = nc.dram_tensor("outT", [3, G], fp32, kind="ExternalOutput")

    dB1 = [nc.dram_tensor(f"dB1_{g}", [N, 128], f16) for g in range(G)]
    dB2 = [nc.dram_tensor(f"dB2_{g}", [N, 128], f16) for g in range(G)]
    dB3 = [nc.dram_tensor(f"dB3_{g}", [N, 128], f16) for g in range(G)]

    with TileContext(nc) as tc:
        import contextlib
        ctx = contextlib.ExitStack()
        with ctx:
            cst = ctx.enter_context(tc.tile_pool(name="cst", bufs=1))
            pg = ctx.enter_context(tc.tile_pool(name="pg", bufs=4))
            paugR = ctx.enter_context(tc.tile_pool(name="paugR", bufs=4))
            pstage = ctx.enter_context(tc.tile_pool(name="pstage", bufs=4))
            patab = ctx.enter_context(tc.tile_pool(name="patab", bufs=4))
            pgath = ctx.enter_context(tc.tile_pool(name="pgath", bufs=2))
            pidx = ctx.enter_context(tc.tile_pool(name="pidx", bufs=3))
            ph = ctx.enter_context(tc.tile_pool(name="ph", bufs=8))
            psm = ctx.enter_context(tc.tile_pool(name="psm", bufs=4))
            psel = ctx.enter_context(tc.tile_pool(name="psel", bufs=3))
            pbig = ctx.enter_context(tc.tile_pool(name="pbig", bufs=1, space="PSUM"))
            pmlp = ctx.enter_context(tc.tile_pool(name="pmlp", bufs=2, space="PSUM"))
            pw = pmlp

            # ---------------- constants / weights
            ident = cst.tile([128, 128], fp32)
            make_identity(nc, ident[:])
            identh = cst.tile([128, 128], f16)
            nc.vector.tensor_copy(identh[:], ident[:])
            half4 = cst.tile([4, 1], f16)
            nc.vector.memset(half4[:], 0.5)
            half64 = cst.tile([64, 1], f16)
            nc.vector.memset(half64[:], 0.5)

            def wload(dram, shape, dt=fp32):
                t = cst.tile(shape, dt, tag=f"w_{dram.name}")
                nc.sync.dma_start(t[:], dram[:])
                return t

            neg1 = cst.tile([1, N], f16)
            nc.vector.memset(neg1[:], -1.0)
            lscr = cst.tile([128, 512], f16)
            T0 = cst.tile([128, 3, G], fp32)
            nc.vector.memset(T0[:], 0.0)
            nc.sync.dma_start(T0[0:4, 2, :], hvals[:])

            # ------------------------------------------------------------------
            def build_srow(augR, xrows, hones, nfeat, augrow):
                """augR[augrow, :] = sum_f xrows[f, :]^2 / 2.

                Stays off the 4-bank score tile: chunks go through 1-bank
                "mlp" PSUM tiles, and the row lands via DMA (engine writes
                must start at a 32-aligned partition)."""
                sq = psm.tile([128, N], f16, tag="sq")
                nc.vector.tensor_tensor(sq[0:nfeat, :], xrows, xrows,
                                        op=ALU.mult)
                stg = psm.tile([1, N], f16, tag="srowstg")
                for c in range(4):
                    sp = pmlp.tile([1, 512], fp32, space="PSUM", tag="mlp")
                    nc.tensor.matmul(sp[:],
                                     lhsT=hones,
                                     rhs=sq[0:nfeat, 512 * c:512 * (c + 1)],
                                     start=True, stop=True)
                    nc.scalar.activation(stg[:, 512 * c:512 * (c + 1)], sp[:],
                                         AF.Copy)
                nc.sync.dma_start(augR[augrow:augrow + 1, :], stg[:])

            def build_atab(w1d_t, xrows, H):
                """A^T [H, N] = w1d^T @ x^T  (feature-major, fp16)."""
                at = patab.tile([H, N], f16, tag="atab")
                for c in range(4):
                    ap = pmlp.tile([H, 512], fp32, space="PSUM", tag="mlp")
                    nc.tensor.matmul(ap[:],
                                     lhsT=w1d_t,
                                     rhs=xrows[:, 512 * c:512 * (c + 1)],
                                     start=True, stop=True)
                    nc.scalar.activation(at[:, 512 * c:512 * (c + 1)], ap[:],
                                         AF.Copy)
                return at

            def build_btab(w1b_t, xrows, H, dB, qlist=None):
                """dB [N, H] token-major fp16 = x @ W1b, written via PSUM->DRAM.
                Batched 4 node-blocks per PSUM tile to shorten the chain.
                qlist selects a subset of the NB//gb pieces so the matmul burst
                can be spread across several pipeline chunks (a lumped burst
                delays the next chunk's score matmuls in the in-order PE queue)."""
                gb = 512 // H          # blocks per psum bank
                for q in qlist if qlist is not None else range(NB // gb):
                    bp = pw.tile([128, gb, H], fp32, space="PSUM", tag="mlp")
                    for b in range(gb):
                        nc.tensor.matmul(
                            bp[:, b, :],
                            lhsT=xrows[:, 128 * (gb * q + b):128 * (gb * q + b + 1)],
                            rhs=w1b_t,
                            start=True, stop=True)
                    stg = psm.tile([128, gb, H], f16, tag="bstg")
                    nc.scalar.activation(stg[:], bp[:], AF.Copy)
                    nc.sync.dma_start(
                        dB[128 * gb * q:128 * gb * (q + 1), :].rearrange(
                            "(b p) h -> p b h", b=gb),
                        stg[:])

            # ------------------------------------------------------------------
            # conv instance: pipelined stages over NCH chunks of CB blocks
            class Inst:
                def __init__(self, augL, augR, at, dB, w2t, b1t, b2t, H,
                             out_rows, HG, late=False, last=False):
                    self.augL = augL
                    self.augR = augR
                    self.at = at
                    self.dB = dB
                    self.w2t = w2t
                    self.b1t = b1t
                    self.b2t = b2t
                    self.H = H
                    self.HG = HG
                    self.out_rows = out_rows
                    self.late = late
                    self.last = last
                    self.idx8fs = [None] * NCH
                    self.gaths = [None] * NCH
                    self.ysbs = [None] * NCH

            def sel_chunk(inst, c4):
                """scores + top-5 selection for CB blocks; writes idx8fs[c4]."""
                idx8f = psm.tile([128, CB, KEC], fp32, tag="idx8f")
                inst.idx8fs[c4] = idx8f
                augL, augR = inst.augL, inst.augR
                for bl in range(CB):
                    b = CB * c4 + bl
                    vp = pbig.tile([128, 2048], fp32, space="PSUM", tag="big")
                    for c in range(4):
                        nc.tensor.matmul(vp[:, 512 * c:512 * (c + 1)],
                                         lhsT=augL[:, 128 * b:128 * (b + 1)],
                                         rhs=augR[:, 512 * c:512 * (c + 1)],
                                         start=True, stop=True)
                    # stage scores to SBUF so the PSUM tile frees early and the
                    # PE can compute the next block's scores during selection
                    sc = psel.tile([128, 2048], f16, tag="sc")
                    if bl == 3 and (c4 % 2 == 1 or inst.late):
                        # DVE has slack post-tournament; siphon 1/8 of the
                        # staging copies off the ACT queue
                        nc.vector.tensor_copy(sc[:], vp[:])
                    else:
                        nc.scalar.activation(sc[:], vp[:], AF.Copy)
                    # 2-round pairwise-max tournament (f16 tensor_tensor runs
                    # in the 2x DVE mode, unlike Max8) reduces the max8 scan
                    # from 2048 to 512 "comb" maxima; max_index then recovers
                    # exact positions from the full tile. Approximation: if
                    # two of a row's true top-5 share a comb (j mod 512), the
                    # weaker is replaced by the next-best outside -- ~2% of
                    # rows swap their 5th neighbor for the 6th.
                    r1 = psel.tile([128, 1024], f16, tag="r1", bufs=2)
                    nc.vector.tensor_tensor(r1[:], sc[:, 0:1024],
                                            sc[:, 1024:2048], op=ALU.max)
                    r2 = psel.tile([128, 512], f16, tag="r2", bufs=2)
                    nc.vector.tensor_tensor(r2[:], r1[:, 0:512],
                                            r1[:, 512:1024], op=ALU.max)
                    r3 = psel.tile([128, 256], f16, tag="r3", bufs=2)
                    nc.vector.tensor_tensor(r3[:], r2[:, 0:256],
                                            r2[:, 256:512], op=ALU.max)
                    vals8 = psm.tile([128, 8], f16, tag="vals8", bufs=8)
                    nc.vector.max(vals8[:], r3[:])
                    iA = psm.tile([128, 8], u16, tag="idxA", bufs=8)
                    nc.vector.max_index(iA[:], vals8[:], sc[:])
                    nc.gpsimd.tensor_copy(idx8f[:, bl, :], iA[:, 0:KEC])

            def gather_chunk(inst, c4):
                idx8f = inst.idx8fs[c4]
                wps = pmlp.tile([16, 8, WPAD], fp32, space="PSUM", tag="mlp")
                for phh in range(8):
                    nc.tensor.matmul(wps[:, phh, 0:CB * KEC],
                                     lhsT=ident[:, 16 * phh:16 * (phh + 1)],
                                     rhs=idx8f[:],
                                     start=True, stop=True)
                wsb = psm.tile([16, CB * 40], fp32, tag="wsb")
                nc.scalar.activation(
                    wsb[:], wps[:, :, 0:CB * KEC].rearrange("q p e -> q e p"),
                    AF.Copy)
                rps = pmlp.tile([128, CB * 40], fp32, space="PSUM", tag="mlp")
                for rc in range(0, CB * 40, 512):
                    re = min(rc + 512, CB * 40)
                    nc.tensor.matmul(rps[:, rc:re], lhsT=repeye_t[:],
                                     rhs=wsb[:, rc:re], start=True, stop=True)
                widx = pidx.tile([128, CB * 40], i16, tag="widx")
                nc.scalar.activation(widx[:], rps[:], AF.Copy)
                gath = pgath.tile([128, 1, CB * KEC * 128], f16, tag="gath")
                nc.gpsimd.dma_gather(
                    out_ap=gath[:], in_ap=inst.dB[:], idxs_ap=widx[:],
                    num_idxs=CB * 128 * KEC, num_idxs_reg=CB * 128 * KEC,
                    elem_size=inst.HG, transpose=True, single_packet=False,
                )
                inst.gaths[c4] = gath

            def mlp_chunk(inst, c4):
                H = inst.H
                gath = inst.gaths[c4]
                inst.ysbs[c4] = ysbs = [None] * CB
                for tl in range(CB):
                    t = CB * c4 + tl
                    zps = pmlp.tile([H, 640], fp32, space="PSUM", tag="mlp")
                    nd = inst.at[:, 128 * t:128 * (t + 1)]
                    nc.tensor.matmul(
                        zps[:, 0:512].rearrange("h (s n) -> h s n", s=4),
                        lhsT=identh[0:H, 0:H],
                        rhs=nd.unsqueeze(1).to_broadcast([H, 4, 128]),
                        start=True, stop=False)
                    nc.tensor.matmul(zps[:, 512:640], lhsT=identh[0:H, 0:H],
                                     rhs=nd, start=True, stop=False)
                    bt = gath[0:H, 0, 640 * tl:640 * (tl + 1)]
                    nc.tensor.matmul(zps[:, 0:512], lhsT=identh[0:H, 0:H],
                                     rhs=bt[:, 0:512], start=False, stop=True)
                    nc.tensor.matmul(zps[:, 512:640], lhsT=identh[0:H, 0:H],
                                     rhs=bt[:, 512:640], start=False, stop=True)
                    ht = ph.tile([H, 640], f16, tag="h")
                    nc.scalar.activation(ht[:], zps[:], AF.Lrelu,
                                         bias=inst.b1t[:], scale=1.0, alpha=0.01)
                    yps = pw.tile([64, 640], fp32, space="PSUM", tag="mlp")
                    nc.tensor.matmul(yps[:, 0:512], lhsT=inst.w2t[:],
                                     rhs=ht[:, 0:512], start=True, stop=True)
                    nc.tensor.matmul(yps[:, 512:640], lhsT=inst.w2t[:],
                                     rhs=ht[:, 512:640], start=True, stop=True)
                    ysb = ph.tile([64, 640], f16, tag="ysb")
                    nc.scalar.activation(ysb[:], yps[:], AF.Lrelu,
                                         bias=inst.b2t[:], scale=1.0, alpha=0.01)
                    ysbs[tl] = ysb

            def reduce_chunk(inst, c4):
                for tl in range(CB):
                    t = CB * c4 + tl
                    y = inst.ysbs[c4][tl]
                    s01 = psm.tile([64, 256], f16, tag="s01")
                    nc.gpsimd.tensor_tensor(s01[:], y[:, 0:256],
                                            y[:, 256:512], op=ALU.add)
                    s0 = psm.tile([64, 128], f16, tag="s23")
                    nc.gpsimd.tensor_tensor(s0[:], s01[:, 0:128],
                                            s01[:, 128:256], op=ALU.add)
                    nc.gpsimd.tensor_tensor(
                        inst.out_rows[:, 128 * t:128 * (t + 1)],
                        s0[:], y[:, 512:640], op=ALU.add)

            # ------------------------------------------------------------------
            # prep pieces (side work between chunks)
            prepped = {}       # g -> (augL1, augR1, a1t)

            def prep_xx_a(g):
                xx_tok = psm.tile([128, NB, 4], fp32, tag="xx_tok")
                nc.sync.dma_start(
                    xx_tok[:],
                    xx_in.rearrange("(gg b p) f -> gg p b f", gg=G, p=128)[g],
                )
                augL1 = pstage.tile([5, N], f16, tag="augL1")
                augR1 = paugR.tile([5, N], f16, tag="augR")
                for c in range(4):
                    xpa = pmlp.tile([4, 512], fp32, space="PSUM", tag="mlp")
                    for b in range(4):
                        nc.tensor.matmul(xpa[:, 128 * b:128 * (b + 1)],
                                         lhsT=xx_tok[:, 4 * c + b, :],
                                         rhs=ident[:],
                                         is_transpose=True,
                                         start=(b == 0), stop=(b == 3))
                    nc.scalar.activation(augL1[0:4, 512 * c:512 * (c + 1)],
                                         xpa[:], AF.Copy)
                nc.vector.tensor_copy(augR1[0:4, :], augL1[0:4, :])
                nc.sync.dma_start(augL1[4:5, :], neg1[:])
                build_srow(augR1, augL1[0:4, :], half4[:], 4, 4)
                prepped[g] = (augL1, augR1, None)

            def prep_xx_b(g):
                augL1, augR1, _ = prepped[g]
                a1t = build_atab(c1w1d_t[:], augL1[0:4, :], 64)
                build_btab(c1w1b_t[:], augL1[0:4, :], 128, dB1[g])
                prepped[g] = (augL1, augR1, a1t)

            prepped2 = {}      # g -> (augR2, a2t) etc.

            def prep_next_a(g, augL, store):
                nc.gpsimd.memset(augL[64:65, :], -1.0)
                augR = paugR.tile([65, N], f16, tag="augR")
                nc.vector.tensor_copy(augR[0:64, :], augL[0:64, :])
                build_srow(augR, augL[0:64, :], half64[:], 64, 64)
                store[g] = (augR, None)

            def prep_next_b(g, augL, dB, store):
                augR, _ = store[g]
                at = build_atab(c2w1d_t[:], augL[0:64, :], 128)
                build_btab(c2w1b_t[:], augL[0:64, :], 128, dB)
                store[g] = (augR, at)

            # ------------------------------------------------------------------
            def lin1_q(g, augL1, augL2, augL3, x3xx, red, q):
                # node-mean commutes past l1w2: accumulate s = sum_nodes
                # Lrelu(feat @ l1w1 + b1) via the ACT accumulator, then the
                # l1w2 matmul runs on [512]-vectors (free size 1) instead of
                # [512, 2048] -- the whole o2 stage collapses
                sl = slice(512 * q, 512 * (q + 1))
                if g == G - 1:
                    # the last graph's lin1 runs after the final selection
                    # chunk: the 4-bank score ring is free, so one [128,4,512]
                    # tile holds all four m-groups and the 12 matmuls pipeline
                    # without touching the busy "mlp" ring
                    lops = pbig.tile([128, 4, 512], fp32, space="PSUM",
                                     tag="big")
                    for m in range(4):
                        nc.tensor.matmul(lops[:, m, :],
                                         lhsT=l1x1_t[:, 128 * m:128 * (m + 1)],
                                         rhs=augL2[0:64, sl],
                                         start=True, stop=False)
                        nc.tensor.matmul(lops[:, m, :],
                                         lhsT=l1x2_t[:, 128 * m:128 * (m + 1)],
                                         rhs=augL3[0:64, sl],
                                         start=False, stop=False)
                        nc.tensor.matmul(lops[:, m, :],
                                         lhsT=l1x3_t[:, 128 * m:128 * (m + 1)],
                                         rhs=x3xx[:, sl],
                                         start=False, stop=True)
                    for m in range(4):
                        nc.scalar.activation(lscr[:], lops[:, m, :], AF.Lrelu,
                                             bias=l1b1_t[:, m:m + 1], scale=1.0,
                                             alpha=0.01,
                                             accum_out=red[:, 4 * m + q:
                                                           4 * m + q + 1])
                    return
                for m in range(4):
                    ops = pmlp.tile([128, 512], fp32, space="PSUM", tag="mlp")
                    nc.tensor.matmul(ops[:], lhsT=l1x1_t[:, 128 * m:128 * (m + 1)],
                                     rhs=augL2[0:64, sl], start=True, stop=False)
                    nc.tensor.matmul(ops[:], lhsT=l1x2_t[:, 128 * m:128 * (m + 1)],
                                     rhs=augL3[0:64, sl], start=False, stop=False)
                    nc.tensor.matmul(ops[:], lhsT=l1x3_t[:, 128 * m:128 * (m + 1)],
                                     rhs=x3xx[:, sl], start=False, stop=True)
                    nc.scalar.activation(lscr[:], ops[:], AF.Lrelu,
                                         bias=l1b1_t[:, m:m + 1], scale=1.0,
                                         alpha=0.01,
                                         accum_out=red[:, 4 * m + q:
                                                       4 * m + q + 1])

            def lin1_fin(g, red):
                s4 = psm.tile([128, 4], f16, tag="s4")
                with nc.allow_low_precision(reason="f16 node-sum feeds f16 l1w2 matmul"):
                    nc.vector.tensor_reduce(
                        s4[:], red[:].rearrange("p (m q) -> p m q", m=4),
                        axis=AX.X, op=ALU.add)
                for mo in range(2):
                    zp = pmlp.tile([128, 1], fp32, space="PSUM", tag="mlp")
                    for k in range(4):
                        nc.tensor.matmul(
                            zp[:],
                            lhsT=l1w2_t[:, k, 128 * mo:128 * (mo + 1)],
                            rhs=s4[:, k:k + 1],
                            start=(k == 0), stop=(k == 3))
                    nc.vector.tensor_scalar(
                        T0[:, mo, g:g + 1], zp[:], 1.0 / N,
                        l1b2_t[:, mo:mo + 1],
                        op0=ALU.mult, op1=ALU.add)

            def final_mlp():
                T0L = cst.tile([128, 3, G], fp32)
                nc.scalar.activation(T0L[:], T0[:], AF.Lrelu, alpha=0.01)
                h1 = cst.tile([128, 2, G], fp32)
                for mo in range(2):
                    zp = pmlp.tile([128, G], fp32, space="PSUM", tag="mlp")
                    for k in range(3):
                        nc.tensor.matmul(zp[:],
                                         lhsT=mw1_t[:, k, 128 * mo:128 * (mo + 1)],
                                         rhs=T0L[:, k, :],
                                         start=(k == 0), stop=(k == 2))
                    nc.scalar.activation(h1[:, mo, :], zp[:], AF.Lrelu,
                                         bias=mb1_t[:, mo:mo + 1], scale=1.0,
                                         alpha=0.01)
                op = pw.tile([3, G], fp32, space="PSUM", tag="mlp")
                for k in range(2):
                    nc.tensor.matmul(op[:], lhsT=mw2_t[:, k, :], rhs=h1[:, k, :],
                                     start=(k == 0), stop=(k == 1))
                osb = cst.tile([3, G], fp32)
                nc.scalar.activation(osb[:], op[:], AF.Identity, bias=mb2_t[:])
                nc.sync.dma_start(outT[:], osb[:])

            # ------------------------------------------------------------------
            # global flat pipeline over NINST instances x NCH chunks
            insts = [None] * NINST
            aL1 = [None] * G
            aL2 = [None] * G
            aL3 = [None] * G
            x3 = [None] * G
            reds = [None] * G

            # graph-0 node features go first so the first score matmul isn't
            # queued behind ~20 weight-load DMAs
            prep_xx_a(0)

            c1w1d_t = wload(c1w1d, [4, 64], f16);  c1w1b_t = wload(c1w1b, [4, 128], f16)
            c1b1_t = wload(c1b1, [64, 1]);  c1w2_t = wload(c1w2, [64, 64], f16)
            c1b2_t = wload(c1b2, [64, 1])
            c2w1d_t = wload(c2w1d, [64, 128], f16);  c2w1b_t = wload(c2w1b, [64, 128], f16)
            c2b1_t = wload(c2b1, [128, 1]);  c2w2_t = wload(c2w2, [128, 64], f16)
            c2b2_t = wload(c2b2, [64, 1])
            l1x1_t = wload(l1x1, [64, 512], f16);  l1x2_t = wload(l1x2, [64, 512], f16)
            l1x3_t = wload(l1x3, [68, 512], f16);  l1b1_t = wload(l1b1, [128, 4])
            l1w2_t = wload(l1w2, [128, 4, 256], f16);  l1b2_t = wload(l1b2, [128, 2])
            mw1_t = wload(mw1, [128, 3, 256]);  mb1_t = wload(mb1, [128, 2])
            mw2_t = wload(mw2, [128, 2, 3]);  mb2_t = wload(mb2, [3, 1])
            repeye_t = wload(repeye, [16, 128])

            prep_xx_b(0)

            def make_inst(i):
                ci, g = divmod(i, G)
                if ci == 0:
                    augL1, augR1, a1t = prepped[g]
                    aL1[g] = augL1
                    t_aL2 = pg.tile([65, N], f16, tag="augL2")
                    aL2[g] = t_aL2
                    return Inst(augL1[:], augR1[:], a1t, dB1[g], c1w2_t,
                                c1b1_t, c1b2_t, 64, t_aL2[0:64, :], HG=128)
                if ci == 1:
                    augR2, a2t = prepped2[g]
                    t_aL3 = pg.tile([65, N], f16, tag="augL3")
                    aL3[g] = t_aL3
                    return Inst(aL2[g][:], augR2[:], a2t, dB2[g], c2w2_t,
                                c2b1_t, c2b2_t, 128, t_aL3[0:64, :], HG=128)
                augR3, a3t = prepped3[g]
                t_x3 = pg.tile([68, N], f16, tag="x3xx")
                x3[g] = t_x3
                return Inst(aL3[g][:], augR3[:], a3t, dB3[g], c2w2_t,
                            c2b1_t, c2b2_t, 128, t_x3[0:64, :], HG=128,
                            late=True, last=(g == G - 1))

            prepped3 = {}

            # side-work table: K -> list of thunks, emitted after sel+gather
            side = {}

            def add_side(k, fn):
                side.setdefault(k, []).append(fn)

            # prep_xx for graphs 1..3 early (graph g needed at K = NCH*g)
            for g in range(1, G):
                add_side(2 * (g - 1), lambda g=g: prep_xx_a(g))
                add_side(2 * (g - 1) + 1, lambda g=g: prep_xx_b(g))
            # pipeline lags: mlp one chunk behind sel (gather DMA ~5.5us has a
            # full ~17.6us chunk to land), reduce one more behind mlp
            LAG_RED = 2
            # prep2(g): after conv1(g) reduces (last at K=NCH*g+NCH-1+LAG_RED);
            # needed at K = NCH*(G+g)
            for g in range(G):
                kbase = NCH * g + NCH + LAG_RED
                add_side(kbase, lambda g=g: prep_next_a(g, aL2[g], prepped2))
                add_side(kbase + 1,
                         lambda g=g: prep_next_b(g, aL2[g], dB2[g], prepped2))
            # prep3(g): after conv2(g)
            for g in range(G):
                kbase = NCH * (G + g) + NCH + LAG_RED
                add_side(kbase, lambda g=g: prep_next_a(g, aL3[g], prepped3))
                add_side(kbase + 1,
                         lambda g=g: prep_next_b(g, aL3[g], dB3[g], prepped3))
            # lin1(g): after conv3(g); x3 holds [x3rows; xx] for the l1x3 matmul
            def lin1_start(g):
                nc.vector.tensor_copy(x3[g][64:68, :], aL1[g][0:4, :])
                red = psm.tile([128, 16], fp32, tag=f"red{g}")
                reds[g] = red

            # lin1_q(q) only reads x3 node-slice 512q:512(q+1) = conv3 chunk
            # q's output, so each piece emits one chunk after that reduce --
            # lin1 spreads over the whole conv3 phase instead of trailing it
            for g in range(G):
                kbase = NCH * (2 * G + g)
                add_side(kbase + LAG_RED, lambda g=g: lin1_start(g))
                for q in range(4):
                    add_side(kbase + q + LAG_RED + 1,
                             lambda g=g, q=q: lin1_q(
                                 g, aL1[g], aL2[g], aL3[g], x3[g], reds[g], q))
                add_side(kbase + NCH + LAG_RED + 1,
                         lambda g=g: lin1_fin(g, reds[g]))

            KTOT = NINST * NCH
            KMAX = KTOT + NCH + 4          # drain window
            for K in range(KMAX):
                if K < KTOT:
                    i, c4 = divmod(K, NCH)
                    if insts[i] is None:
                        insts[i] = make_inst(i)
                    sel_chunk(insts[i], c4)
                    gather_chunk(insts[i], c4)
                for fn in side.pop(K, ()):
                    fn()
                Kr = K - 2
                if 0 <= Kr < KTOT:
                    i, c4 = divmod(Kr, NCH)
                    reduce_chunk(insts[i], c4)
                Km = K - 1
                if 0 <= Km < KTOT:
                    i, c4 = divmod(Km, NCH)
                    mlp_chunk(insts[i], c4)

            final_mlp()

    nc.compile()
    return nc


def _get_module():
    global _MODULE
    if _MODULE is None:
        _MODULE = _build_module()
    return _MODULE


# --------------------------------------------------------------------------
# host: exact homophily (duplicate values only; usually all-zero)
# --------------------------------------------------------------------------
def _homophily_host(xx):
    xx = xx.reshape(B_TOTAL, N, 4)
    h = np.zeros((B_TOTAL, 4), np.float32)
    for b in range(B_TOTAL):
        xg = xx[b].astype(np.float32)
        s = None
        for c in range(4):
            vals = xg[:, c]
            u, inv, cnt = np.unique(vals, return_inverse=True,
                                    return_counts=True)
            if (cnt <= 1).all():
                continue
            if s is None:
                s = (xg.astype(np.float32) ** 2).sum(axis=1)
            for ui in np.where(cnt > 1)[0]:
                nodes = np.where(inv == ui)[0]
                for i in nodes:
                    d = s + s[i] - 2.0 * (xg @ xg[i])
                    d = d.astype(np.float32)
                    d[i] = d[i] + np.float32(1e9)
                    for j in nodes:
                        if j == i:
                            continue
                        rank = int((d < d[j]).sum()) + int(
                            ((d == d[j]) & (np.arange(N) < j)).sum())
                        if rank < 50:
                            h[b, c] += 1.0
    return h / np.float32(N * 50.0)


# --------------------------------------------------------------------------
# host entry point
# --------------------------------------------------------------------------
def _prepare_in_maps(inputs):
    x = np.ascontiguousarray(np.asarray(inputs["x"], np.float32))
    pos = np.ascontiguousarray(np.asarray(inputs["pos"], np.float32))
    w = {k: np.ascontiguousarray(np.asarray(inputs[k], np.float32)) for k in
         ("c1w1", "c1b1", "c1w2", "c1b2", "c2w1", "c2b1", "c2w2", "c2b2",
          "l1w1", "l1b1", "l1w2", "l1b2", "mw1", "mb1", "mw2", "mb2")}

    xx = np.concatenate([x, pos], axis=1)                      # [B*N, 4]
    hv = _homophily_host(xx)                                   # [32, 4]

    def f16(a):
        return np.ascontiguousarray(np.asarray(a, np.float16))

    # conv weight decompositions
    c1w1 = w["c1w1"]
    c1w1d = f16(c1w1[0:4] - c1w1[4:8])
    c1w1b = np.zeros((4, 128), np.float16)
    c1w1b[:, 0:64] = f16(c1w1[4:8])
    c1w1b = np.ascontiguousarray(c1w1b)
    c2w1 = w["c2w1"]
    c2w1d = f16(c2w1[0:64] - c2w1[64:128])
    c2w1b = f16(c2w1[64:128])

    # lin1 rows reordered to [x1, x2, x3, xx]
    l1w1 = w["l1w1"]                                           # [196, 512]
    l1x1 = f16(l1w1[4:68])
    l1x2 = f16(l1w1[68:132])
    l1x3 = f16(np.concatenate([l1w1[132:196], l1w1[0:4]], axis=0))
    l1b1c = np.ascontiguousarray(w["l1b1"].reshape(4, 128).T)  # [128, 4]
    l1w2c = f16(w["l1w2"].reshape(4, 128, 256).transpose(1, 0, 2))
    l1b2c = np.ascontiguousarray(w["l1b2"].reshape(2, 128).T)  # [128, 2]
    mw1p = np.zeros((384, 256), np.float32)
    mw1p[0:260] = w["mw1"]
    mw1c = np.ascontiguousarray(mw1p.reshape(3, 128, 256).transpose(1, 0, 2))
    mb1c = np.ascontiguousarray(w["mb1"].reshape(2, 128).T)    # [128, 2]
    mw2c = np.ascontiguousarray(w["mw2"].reshape(2, 128, 3).transpose(1, 0, 2))
    mb2c = np.ascontiguousarray(w["mb2"].reshape(3, 1))

    shared = dict(
        c1w1d=c1w1d, c1w1b=c1w1b, c1b1=w["c1b1"].reshape(64, 1),
        c1w2=f16(w["c1w2"]), c1b2=w["c1b2"].reshape(64, 1),
        c2w1d=c2w1d, c2w1b=c2w1b, c2b1=w["c2b1"].reshape(128, 1),
        c2w2=f16(w["c2w2"]), c2b2=w["c2b2"].reshape(64, 1),
        l1x1=l1x1, l1x2=l1x2, l1x3=l1x3, l1b1=l1b1c,
        l1w2=l1w2c, l1b2=l1b2c,
        mw1=mw1c, mb1=mb1c, mw2=mw2c, mb2=mb2c,
        repeye=np.ascontiguousarray(np.tile(np.eye(16, dtype=np.float32), 8)),
    )

    in_maps = []
    for c in range(NCORES):
        im = dict(shared)
        im["xx"] = np.ascontiguousarray(xx[G * N * c:G * N * (c + 1)])
        im["hvals"] = np.ascontiguousarray(hv[G * c:G * (c + 1)].T)  # [4ch, G]
        in_maps.append(im)
    return in_maps


def kernel(**inputs):
    in_maps = _prepare_in_maps(inputs)
    nc = _get_module()
    from concourse.bass_utils import run_bass_kernel_spmd
    res = run_bass_kernel_spmd(nc, in_maps, list(range(NCORES)))
    out = np.concatenate([r["outT"].T for r in res.results], axis=0)  # [32, 3]
    return np.ascontiguousarray(out.astype(np.float32))
